# revision 33
# baseline (speedup 1.0000x reference)
"""GAT-style attention-diagonal kernel for Trainium2 (Bass/Tile), 8-core SPMD.

Reference computation (per (b,t) slice, x:[N,F]):
    Q = x@Wq + bq; K = x@Wk + bk; V = x@Wv + bv
    s = Q @ K.T / sqrt(F)            # [N,N]
    a = softmax(s, axis=-1)
    out = diag(a)[:, None] * V       # only the softmax diagonal is needed

Sharding: data-parallel on the fused B*T axis (48 slices -> 6 per core),
weights replicated.

Per-core per-slice dataflow (fused path, bq=bk=0):
  - load x tile [1024,512], PE-transpose to XT [f,n] (32x 128x128 transposes)
  - one-time M = Wq @ Wk.T on device; then scores = (X M) X.T needs only
    XMT[g,n] = M.T @ XT per slice (the K projection is eliminated)
  - V[n,g] = XT.T-stationary @ Wv (natural layout for the final row scale)
  - scores rows: per 128-row chunk: XMT-chunk stationary x XT streaming
    -> two PSUM banks [128,512]; exp fused with row-sum on ScalarE
    (no max-subtraction: scaled scores are ~N(0,1), exp cannot overflow);
    diagonal of exp(scores) via identity-mask + reduce on the exp'd bank
  - diag = exp_nn / rowsum;  out_chunk = diag * V_chunk

All matmuls run in fp32r (full-rate 4-byte PE path, ~1e-4 rel err); PSUM
accumulation is fp32.
"""

import numpy as np

B, T, N, F = 4, 12, 1024, 512
NCORES = 8
S = (B * T) // NCORES  # 6 slices per core
P = 128
NO = N // P   # 8 row chunks per slice
FO = F // P   # 4 f chunks
GO = F // P   # 4 g chunks
MH = N // 512  # 2 halves of the scores free axis
SCALE = float(1.0 / np.sqrt(np.float32(F)))

_CACHE: dict = {}


def build_program(n_slices: int = S, repeats: int = 1, fused_qk: bool = True):
    import concourse.bass as bass
    import concourse.tile as tile
    from concourse import bacc, mybir
    from concourse.masks import make_identity
    from contextlib import ExitStack

    f32 = mybir.dt.float32
    f32r = mybir.dt.float32r
    EXP = mybir.ActivationFunctionType.Exp
    COPYF = mybir.ActivationFunctionType.Identity
    AX = mybir.AxisListType.X
    OP = mybir.AluOpType

    nc = bacc.Bacc(trn_type="TRN2", target_bir_lowering=False, debug=False)
    x_d = nc.dram_tensor("x", [n_slices, N, F], f32, kind="ExternalInput").ap()
    wq_d = nc.dram_tensor("wq", [F, F], f32, kind="ExternalInput").ap()
    wk_d = nc.dram_tensor("wk", [F, F], f32, kind="ExternalInput").ap()
    wv_d = nc.dram_tensor("wv", [F, F], f32, kind="ExternalInput").ap()
    bq_d = nc.dram_tensor("bq", [F], f32, kind="ExternalInput").ap()
    bk_d = nc.dram_tensor("bk", [F], f32, kind="ExternalInput").ap()
    bv_d = nc.dram_tensor("bv", [F], f32, kind="ExternalInput").ap()
    out_d = nc.dram_tensor("out", [n_slices, N, F], f32, kind="ExternalOutput").ap()

    with tile.TileContext(nc) as tc, ExitStack() as ctx:
        consts = ctx.enter_context(tc.tile_pool(name="consts", bufs=1))
        stage = ctx.enter_context(tc.tile_pool(name="stage", bufs=1))
        xin_pool = ctx.enter_context(tc.tile_pool(name="xin", bufs=2))
        xt_pool = ctx.enter_context(tc.tile_pool(name="xt", bufs=2))
        proj_pool = ctx.enter_context(
            tc.tile_pool(name="proj", bufs=2 if fused_qk else 1)
        )
        outp = ctx.enter_context(tc.tile_pool(name="outp", bufs=3))
        dscr = ctx.enter_context(tc.tile_pool(name="dscr", bufs=2))
        stats = ctx.enter_context(tc.tile_pool(name="stats", bufs=6))
        # PSUM budget: 8 banks total = pp(2) + sp(2+2) + tp(2)
        pp = ctx.enter_context(tc.tile_pool(name="pp", bufs=2, space="PSUM"))
        sp = ctx.enter_context(tc.tile_pool(name="sp", bufs=2, space="PSUM"))
        tp = ctx.enter_context(tc.tile_pool(name="tp", bufs=2, space="PSUM"))

        ident = consts.tile([P, P], f32, name="ident", tag="ident")
        make_identity(nc, ident[:])

        # weights staged as fp32 then rounded to fp32r (the PE's fast 4-byte
        # format; the BIR verifier requires producers to round explicitly)
        w_stages = {}
        w_sbs = {}
        for nm, wd in (("wq", wq_d), ("wk", wk_d), ("wv", wv_d)):
            w_stage = stage.tile([P, FO, F], f32, name=f"{nm}_stage", tag=f"{nm}_stage")
            nc.sync.dma_start(w_stage[:], wd.rearrange("(fo fi) g -> fi fo g", fi=P))
            w_stages[nm] = w_stage
            if nm == "wv" or not fused_qk:
                w_sb = consts.tile([P, FO, F], f32r, name=f"{nm}_sb", tag=f"{nm}_sb")
                nc.vector.tensor_copy(w_sb[:], w_stage[:])
                w_sbs[nm] = w_sb
        wv_sb = w_sbs["wv"]

        # biases: bq/bk striped per-partition [gi, go]; bv broadcast to all
        # partitions (it adds along the free axis of V's natural layout)
        bq_sb = consts.tile([P, GO], f32, name="bq_sb", tag="bq_sb")
        nc.sync.dma_start(bq_sb[:], bq_d.rearrange("(go gi) -> gi go", gi=P))
        bk_sb = consts.tile([P, GO], f32, name="bk_sb", tag="bk_sb")
        nc.sync.dma_start(bk_sb[:], bk_d.rearrange("(go gi) -> gi go", gi=P))
        bv_bc = consts.tile([P, F], f32, name="bv_bc", tag="bv_bc")
        nc.sync.dma_start(bv_bc[:], bv_d.unsqueeze(0).to_broadcast((P, F)))

        if fused_qk:
            # one-time M = Wq @ Wk.T (f32r), stored like a weight [ai, ao, b].
            # Needs Wq^T and Wk^T chunks first (PE transpose via identity);
            # 4 transposes share one PSUM bank -> one wide strided copy.
            wt_sbs = {}
            for nm in ("wq", "wk"):
                wt_sb = consts.tile([P, FO, F], f32r, name=f"{nm}t_sb", tag=f"{nm}t_sb")
                for ao in range(FO):
                    t_ps = tp.tile([P, FO, P], f32, name="t_ps", tag="t_ps")
                    for co in range(FO):
                        nc.tensor.transpose(
                            t_ps[:, co],
                            w_stages[nm][:, ao, co * P : (co + 1) * P],
                            ident[:],
                        )
                    nc.vector.tensor_copy(
                        wt_sb[:, :, ao * P : (ao + 1) * P], t_ps[:]
                    )
                wt_sbs[nm] = wt_sb
            m_sb = consts.tile([P, FO, F], f32r, name="m_sb", tag="m_sb")
            for ao in range(FO):
                ps = pp.tile([P, F], f32, name="ps_proj", tag="ps_proj")
                for co in range(FO):
                    nc.tensor.matmul(
                        ps[:],
                        wt_sbs["wq"][:, co, ao * P : (ao + 1) * P],
                        wt_sbs["wk"][:, co, :],
                        start=(co == 0),
                        stop=(co == FO - 1),
                    )
                nc.vector.tensor_copy(m_sb[:, ao, :], ps[:])

        for s in [sl for _ in range(repeats) for sl in range(n_slices)]:
            # ---- load x and transpose to XT [fi, fo, n] ----
            x_sb = xin_pool.tile([P, NO, F], f32, name="x_sb", tag="x_sb")
            x_r = x_d[s].rearrange("(no p) f -> p no f", p=P)
            # per-row-chunk loads so each transpose group starts as soon as
            # its own 256KB chunk lands
            for no in range(NO):
                nc.sync.dma_start(x_sb[:, no : no + 1], x_r[:, no : no + 1])

            xt_sb = xt_pool.tile([P, FO, N], f32r, name="xt_sb", tag="xt_sb")
            for no in range(NO):
                t_ps = tp.tile([P, FO, P], f32, name="t_ps", tag="t_ps")
                for fo in range(FO):
                    nc.tensor.transpose(
                        t_ps[:, fo], x_sb[:, no, fo * P : (fo + 1) * P], ident[:]
                    )
                nc.vector.tensor_copy(
                    xt_sb[:, :, no * P : (no + 1) * P], t_ps[:]
                )

            # ---- projections ----
            if fused_qk:
                # XMT[b, n] = sum_a M[a, b] * XT[a, n]; scores lhsT source.
                # V is computed per-row-chunk inside the scores loop (its
                # PSUM tile is scaled directly, no SBUF copy).
                qt_sb = proj_pool.tile([P, GO, N], f32r, name="qt_sb", tag="qt_sb")
                kt_sb = xt_sb  # scores stream directly against X^T
                proj_list = ((m_sb, None, qt_sb, True),)
            else:
                v_sb = proj_pool.tile([P, NO, F], f32, name="v_sb", tag="v_sb")
                qt_sb = proj_pool.tile([P, GO, N], f32r, name="qt_sb", tag="qt_sb")
                kt_sb = proj_pool.tile([P, GO, N], f32r, name="kt_sb", tag="kt_sb")
                proj_list = (
                    (w_sbs["wq"], bq_sb, qt_sb, True),
                    (w_sbs["wk"], bk_sb, kt_sb, False),
                )

            for w_sb, b_sb, dst, on_act in proj_list:
                for go in range(GO):
                    for nh in range(MH):
                        ps = pp.tile([P, 512], f32, name="ps_proj", tag="ps_proj")
                        for fo in range(FO):
                            nc.tensor.matmul(
                                ps[:],
                                w_sb[:, fo, go * P : (go + 1) * P],
                                xt_sb[:, fo, nh * 512 : (nh + 1) * 512],
                                start=(fo == 0),
                                stop=(fo == FO - 1),
                            )
                        # PSUM->SBUF copy (+bias) + round-to-f32r; Q on the
                        # (otherwise idle) scalar engine, K on DVE
                        if on_act:
                            if b_sb is None:
                                nc.scalar.activation(
                                    dst[:, go, nh * 512 : (nh + 1) * 512], ps[:], COPYF
                                )
                            else:
                                nc.scalar.activation(
                                    dst[:, go, nh * 512 : (nh + 1) * 512],
                                    ps[:],
                                    COPYF,
                                    bias=b_sb[:, go : go + 1],
                                )
                        else:
                            nc.vector.tensor_scalar_add(
                                dst[:, go, nh * 512 : (nh + 1) * 512],
                                ps[:],
                                b_sb[:, go : go + 1],
                            )

            if not fused_qk:
                for no in range(NO):
                    ps = pp.tile([P, F], f32, name="ps_proj", tag="ps_proj")
                    for fo in range(FO):
                        nc.tensor.matmul(
                            ps[:],
                            xt_sb[:, fo, no * P : (no + 1) * P],
                            wv_sb[:, fo, :],
                            start=(fo == 0),
                            stop=(fo == FO - 1),
                        )
                    nc.vector.tensor_add(v_sb[:, no, :], ps[:], bv_bc[:])

            # ---- scores / softmax-diag / output ----
            for no in range(NO):
                ps0 = sp.tile([P, 512], f32, name="ps_s0", tag="ps_s0")
                ps1 = sp.tile([P, 512], f32, name="ps_s1", tag="ps_s1")
                for go in range(GO):
                    lhsT = qt_sb[:, go, no * P : (no + 1) * P]
                    nc.tensor.matmul(
                        ps0[:], lhsT, kt_sb[:, go, 0:512],
                        start=(go == 0), stop=(go == GO - 1),
                    )
                    nc.tensor.matmul(
                        ps1[:], lhsT, kt_sb[:, go, 512:1024],
                        start=(go == 0), stop=(go == GO - 1),
                    )
                if fused_qk:
                    # V row-chunk just-in-time; scaled straight from PSUM
                    ps_v = pp.tile([P, F], f32, name="ps_proj", tag="ps_proj")
                    for fo in range(FO):
                        nc.tensor.matmul(
                            ps_v[:],
                            xt_sb[:, fo, no * P : (no + 1) * P],
                            wv_sb[:, fo, :],
                            start=(fo == 0),
                            stop=(fo == FO - 1),
                        )

                # no max-subtraction: scaled scores are ~N(0,1), exp cannot
                # overflow fp32 (|s*scale| stays well under ~10)
                s0 = stats.tile([P, 1], f32, name="s0", tag="s0")
                s1 = stats.tile([P, 1], f32, name="s1", tag="s1")
                nc.scalar.activation(
                    ps0[:], ps0[:], EXP, scale=SCALE, accum_out=s0[:]
                )
                nc.scalar.activation(
                    ps1[:], ps1[:], EXP, scale=SCALE, accum_out=s1[:]
                )
                # diagonal of exp(scores) via identity mask on the exp'd bank
                bank, off = divmod(no * P, 512)
                psd = ps0 if bank == 0 else ps1
                dblk = dscr.tile([P, P], f32, name="dblk", tag="dblk")
                nc.vector.tensor_mul(dblk[:], psd[:, off : off + P], ident[:])
                snn = stats.tile([P, 1], f32, name="snn", tag="snn")
                nc.vector.tensor_reduce(snn[:], dblk[:], axis=AX, op=OP.add)

                ssum = stats.tile([P, 1], f32, name="ssum", tag="ssum")
                nc.vector.tensor_add(ssum[:], s0[:], s1[:])
                rec = stats.tile([P, 1], f32, name="rec", tag="rec")
                nc.vector.reciprocal(rec[:], ssum[:])
                dval = stats.tile([P, 1], f32, name="dval", tag="dval")
                nc.vector.tensor_mul(dval[:], snn[:], rec[:])

                ot = outp.tile([P, F], f32, name="ot", tag="ot")
                if fused_qk:
                    nc.vector.tensor_scalar_mul(ot[:], ps_v[:], dval[:])
                else:
                    nc.vector.tensor_scalar_mul(ot[:], v_sb[:, no, :], dval[:])
                nc.sync.dma_start(out_d[s, no * P : (no + 1) * P, :], ot[:])

    nc.compile()
    return nc


def _get_runner(fused: bool):
    """Build the Bass program once and wrap it in a cached jitted shard_map
    dispatcher (mirrors bass2jax.run_bass_via_pjrt, minus donation so the
    pre-zeroed output operands can be reused across calls — this kernel
    writes every output element)."""
    key = ("runner", fused)
    if key in _CACHE:
        return _CACHE[key]

    import jax
    from jax.experimental.shard_map import shard_map
    from jax.sharding import Mesh, NamedSharding, PartitionSpec
    from concourse import mybir
    from concourse.bass2jax import (
        _bass_exec_p,
        install_neuronx_cc_hook,
        partition_id_tensor,
    )

    nc = build_program(S, fused_qk=fused)
    install_neuronx_cc_hook()
    partition_name = nc.partition_id_tensor.name if nc.partition_id_tensor else None

    in_names, out_names, out_avals, zero_outs = [], [], [], []
    for alloc in nc.m.functions[0].allocations:
        if not isinstance(alloc, mybir.MemoryLocationSet):
            continue
        name = alloc.memorylocations[0].name
        if alloc.kind == "ExternalInput":
            if name != partition_name:
                in_names.append(name)
        elif alloc.kind == "ExternalOutput":
            shape = tuple(alloc.tensor_shape)
            np_dt = mybir.dt.np(alloc.dtype)
            out_avals.append(jax.core.ShapedArray(shape, np_dt))
            out_names.append(name)
            zero_outs.append(np.zeros(shape, np_dt))

    n_params = len(in_names)
    all_in_names = list(in_names) + list(out_names)
    if partition_name is not None:
        all_in_names.append(partition_name)

    def _body(*args):
        operands = list(args)
        if partition_name is not None:
            operands.append(partition_id_tensor())
        outs = _bass_exec_p.bind(
            *operands,
            out_avals=tuple(out_avals),
            in_names=tuple(all_in_names),
            out_names=tuple(out_names),
            lowering_input_output_aliases=(),
            sim_require_finite=True,
            sim_require_nnan=True,
            nc=nc,
        )
        return tuple(outs)

    devices = jax.devices()[:NCORES]
    mesh = Mesh(np.asarray(devices), ("core",))
    n_outs = len(out_names)
    fn = jax.jit(
        shard_map(
            _body,
            mesh=mesh,
            in_specs=(PartitionSpec("core"),) * (n_params + n_outs),
            out_specs=(PartitionSpec("core"),) * n_outs,
            check_rep=False,
        ),
        keep_unused=True,
    )
    sharding = NamedSharding(mesh, PartitionSpec("core"))
    concat_zeros = [
        jax.device_put(
            np.zeros((NCORES * z.shape[0], *z.shape[1:]), z.dtype), sharding
        )
        for z in zero_outs
    ]
    runner = {
        "fn": fn,
        "in_names": in_names,
        "out_names": out_names,
        "zeros": concat_zeros,
        "sharding": sharding,
    }
    _CACHE[key] = runner
    return runner


def kernel(x, Wq, bq, Wk, bk, Wv, bv):
    import jax

    x = np.ascontiguousarray(np.asarray(x, dtype=np.float32))
    shards = x.reshape(B * T, N, F).reshape(NCORES, S, N, F)

    bq = np.ascontiguousarray(np.asarray(bq, dtype=np.float32))
    bk = np.ascontiguousarray(np.asarray(bk, dtype=np.float32))
    bv_arr = np.ascontiguousarray(np.asarray(bv, dtype=np.float32))
    # the fused path assumes zero biases (scores = X (Wq Wk^T) X^T and V
    # scaled straight from PSUM); fall back to the general path otherwise
    fused = bool(not bq.any() and not bk.any() and not bv_arr.any())

    runner = _get_runner(fused)

    per_core = {
        "x": shards.reshape(NCORES * S, N, F),
        "wq": np.tile(np.asarray(Wq, np.float32)[None], (NCORES, 1, 1)).reshape(
            NCORES * F, F
        ),
        "wk": np.tile(np.asarray(Wk, np.float32)[None], (NCORES, 1, 1)).reshape(
            NCORES * F, F
        ),
        "wv": np.tile(np.asarray(Wv, np.float32)[None], (NCORES, 1, 1)).reshape(
            NCORES * F, F
        ),
        "bq": np.tile(bq, NCORES),
        "bk": np.tile(bk, NCORES),
        "bv": np.tile(bv_arr, NCORES),
    }
    def _run(r):
        args = [
            jax.device_put(np.ascontiguousarray(per_core[nm]), r["sharding"])
            for nm in r["in_names"]
        ]
        outs = r["fn"](*args, *r["zeros"])
        return np.asarray(outs[r["out_names"].index("out")])

    try:
        out = _run(runner)
    except Exception:
        # stale cached executable/buffers (e.g. device session reset
        # between calls): rebuild once and retry
        _CACHE.pop(("runner", fused), None)
        out = _run(_get_runner(fused))
    return out.reshape(B, T, N, F)
